# revision 19
# baseline (speedup 1.0000x reference)
"""Multihead attention (B=4, S=2048, D=1024, H=16) on 8 Trainium2 NeuronCores.

Sharding: data-parallel over batch (4) x tensor-parallel over heads (2 groups
of 8 heads). Core c handles batch c//2, head-group c%2. Q/K/V projections are
column-parallel (each core owns 512 rows of Wq/Wk/Wv), attention is fully
local per head, out-projection is row-parallel (each core owns 512 columns of
Wo) producing a partial [S, D] output; the two partials per batch are summed
on the host (the "all-reduce").

Device layout (per core, all bf16 unless noted):
  xqT/xkT/xvT [1024, 2048]  = x[b].T            (host-transposed, bf16)
  wqT/wkT/wvT [1024, 512]   = W[g*512:,:].T
  woT         [512, 1024]   = Wo[:, g*512:].T
  bq/bk [128, 4] f32 (partition-major), bv [1, 512] f32
  outp        [2048, 1024] bf16 (partial output; summed in f32 on host)

On-chip schedule: heads are processed in PAIRS (2p, 2p+1) whose K/Q slices
sit at SBUF partitions 0-63 / 64-127 of chunk p, so the two scores matmuls
(contraction 64) of a pair occupy disjoint row-halves of the PE array
(tile_position (0,0)/(64,0)) and stream concurrently on hardware.

Iteration (j, p): q-tile j (512 q), pair p; inner loop over 16 k-chunks:
  S^T [128k, 1024] = [K_h Q_h^T | K_h' Q_h'^T]  (row-tiled pair, PSUM)
  attn^T = exp(S^T/8) -> SBUF bf16               (ScalarE, the serial floor)
  O'[65, 512] += [V_h | 1]^T attn_h^T  (x2 heads, PSUM accumulate over kc)
Scores are emitted two k-chunks ahead of AV so the PE never waits on the
ScalarE exp. All projection work (K chunks 1-3, Q groups, V groups) and the
out-projection of completed q-tiles are interleaved into the attention loop
as PE filler, so the tensor engine stays continuously busy (which also keeps
it at the 2.4 GHz p-state) and the kernel has no serial projection phases;
only the first iteration (V-projection JIT) and the last q-tile's
out-projection fall outside the steady state.
"""

import sys

if "/opt/trn_rl_repo" not in sys.path:
    sys.path.insert(0, "/opt/trn_rl_repo")

import numpy as np
import ml_dtypes

P = 128
S = 2048
DIN = 1024
DG = 512          # per-core projection width (8 heads * 64)
HD = 64
NH_LOCAL = 8      # heads per core
N_CORES = 8
VA = NH_LOCAL * (HD + 1)  # V_aug free width (520)

_CACHE: dict = {}


def build_bass(repeat: int = 1):
    """Build the SPMD single-core program (same program on all 8 cores)."""
    from concourse import bacc, tile, mybir

    f32 = mybir.dt.float32
    bf16 = mybir.dt.bfloat16

    nc = bacc.Bacc("TRN2", target_bir_lowering=False, debug=False,
                   num_devices=N_CORES)

    xqT = nc.dram_tensor("xqT", [DIN, S], bf16, kind="ExternalInput")
    xkT = nc.dram_tensor("xkT", [DIN, S], bf16, kind="ExternalInput")
    xvT = nc.dram_tensor("xvT", [DIN, S], bf16, kind="ExternalInput")
    wqT = nc.dram_tensor("wqT", [DIN, DG], bf16, kind="ExternalInput")
    wkT = nc.dram_tensor("wkT", [DIN, DG], bf16, kind="ExternalInput")
    wvT = nc.dram_tensor("wvT", [DIN, DG], bf16, kind="ExternalInput")
    woT = nc.dram_tensor("woT", [DG, DIN], bf16, kind="ExternalInput")
    bqd = nc.dram_tensor("bq", [P, 4], f32, kind="ExternalInput")
    bkd = nc.dram_tensor("bk", [P, 4], f32, kind="ExternalInput")
    bvd = nc.dram_tensor("bv", [1, DG], f32, kind="ExternalInput")
    outp = nc.dram_tensor("outp", [S, DIN], bf16, kind="ExternalOutput")

    with tile.TileContext(nc) as tc:
        for _ in range(repeat):
            _emit(nc, tc, xqT, xkT, xvT, wqT, wkT, wvT, woT, bqd, bkd, bvd,
                  outp)
    nc.compile()
    return nc


def _emit(nc, tc, xqT, xkT, xvT, wqT, wkT, wvT, woT, bqd, bkd, bvd, outp):
    from concourse import mybir

    f32 = mybir.dt.float32
    bf16 = mybir.dt.bfloat16
    Exp = mybir.ActivationFunctionType.Exp
    mult = mybir.AluOpType.mult
    add_op = mybir.AluOpType.add

    with (
        tc.tile_pool(name="consts", bufs=1) as consts,
        tc.tile_pool(name="xin", bufs=3) as xin,
        tc.tile_pool(name="qkv", bufs=1) as qkvp,
        tc.tile_pool(name="attn", bufs=4) as attnp,
        tc.tile_pool(name="small", bufs=2) as smallp,
        tc.tile_pool(name="osb", bufs=3) as osbp,
        tc.tile_pool(name="ps", bufs=2, space="PSUM") as psp,
        tc.tile_pool(name="pav", bufs=2, space="PSUM") as pav,
        tc.tile_pool(name="pss", bufs=2, space="PSUM") as pss,
    ):
        # PSUM budget (8 banks): "s" [128,1024]f32 x2 = 4 banks (paired
        # scores), "av" [65,512]f32 x2 = 2 banks, "qp" [128,512]f32 x2 =
        # 2 banks (projection + out-proj groups).
        QT = qkvp.tile([P, 4, S], bf16, tag="QT")
        KT = qkvp.tile([P, 4, S], bf16, tag="KT")
        vaug = qkvp.tile([P, 16, VA], bf16, tag="vaug")
        OT = qkvp.tile([P, 4, S], bf16, tag="OT")

        # ones columns of V_aug
        for h in range(NH_LOCAL):
            nc.vector.memset(vaug[:, :, h * (HD + 1) + HD], 1.0)

        def load_x(xdram, tag):
            # x^T [DIN, S] -> two SBUF tiles [P, 4, S]; each half split over
            # two DMA queues (sync + gpsimd) to halve load latency. xk/xq
            # share a 2-slot ring (K projection completes before any Q-proj
            # emission, so the WAR never crosses the attention loop); xv has
            # its own ring because V groups are deferred into iteration 0.
            slot_tag = "xv" if tag == "v" else "xkq"
            xt = xdram.ap().rearrange("(h c p) m -> h p c m", h=2, p=P)
            halves = []
            for hhalf in range(2):
                xh = xin.tile([P, 4, S], bf16, tag=slot_tag, bufs=2)
                nc.sync.dma_start(xh[:, 0:2], xt[hhalf, :, 0:2])
                nc.gpsimd.dma_start(xh[:, 2:4], xt[hhalf, :, 2:4])
                halves.append(xh)
            return halves

        def proj_inputs(xdram, wdram, bdram, wtag):
            bias = consts.tile([P, 4], f32, tag=f"b_{wtag}")
            nc.sync.dma_start(bias[:], bdram.ap())
            w = consts.tile([P, 8, DG], bf16, tag=f"w_{wtag}")
            nc.sync.dma_start(w[:], wdram.ap().rearrange("(c p) m -> p c m",
                                                         p=P))
            halves = load_x(xdram, wtag)
            return w, bias, halves

        def proj_group(w, bias, halves, dstT, c, st):
            # dstT[dq, s] for dq chunk c, s-tile st (one PSUM group)
            pt = psp.tile([P, 512], f32, tag="qp", name=f"pj_{c}_{st}")
            for kc in range(8):
                nc.tensor.matmul(
                    pt[:],
                    w[:, kc, c * P:(c + 1) * P],
                    halves[kc // 4][:, kc % 4, st * 512:(st + 1) * 512],
                    start=(kc == 0),
                    stop=(kc == 7),
                )
            nc.vector.tensor_scalar_add(
                dstT[:, c, st * 512:(st + 1) * 512], pt[:],
                bias[:, c:c + 1])

        # ---- input DMAs + K projection, fully before the xq load so the
        # shared xin ring (bufs=3) never creates a PE-order cycle: xq's slot
        # WAR depends on the K-projection matmuls, which all precede any
        # attention work in PE program order.
        wk, bk, xkh = proj_inputs(xkT, wkT, bkd, "k")

        bvrow = consts.tile([1, DG], f32, tag="bvrow")
        nc.sync.dma_start(bvrow[:], bvd.ap())
        bvb = consts.tile([P, DG], f32, tag="bvb")
        nc.gpsimd.partition_broadcast(bvb[:], bvrow[:])
        bvb3 = bvb[:].rearrange("p (h f) -> p h f", f=HD)
        wv = consts.tile([P, 8, DG], bf16, tag="w_v")
        nc.sync.dma_start(wv[:], wvT.ap().rearrange("(c p) m -> p c m", p=P))
        xvh = load_x(xvT, "v")

        wq, bq, xqh = proj_inputs(xqT, wqT, bqd, "q")

        wo = consts.tile([P, 4, DIN], bf16, tag="wo")
        nc.sync.dma_start(wo[:], woT.ap().rearrange("(c p) m -> p c m", p=P))

        def v_group(sc):
            # V projected directly in [s, dv] layout into V_aug columns,
            # bias added via a partition-broadcast row.
            pt = psp.tile([P, DG], f32, tag="qp", name=f"pv_{sc}")
            for kc in range(8):
                nc.tensor.matmul(
                    pt[:],
                    xvh[kc // 4][:, kc % 4, sc * P:(sc + 1) * P],
                    wv[:, kc, :],
                    start=(kc == 0), stop=(kc == 7),
                )
            dst3 = vaug[:, sc].rearrange("p (h f) -> p h f",
                                         f=HD + 1)[:, :, 0:HD]
            src3 = pt[:].rearrange("p (h f) -> p h f", f=HD)
            nc.vector.tensor_tensor(dst3, src3, bvb3, add_op)

        def oproj_group(st, nh):
            # partial[s, dout] = sum_dq OT[dq, s] * woT[dq, dout]
            po = psp.tile([P, 512], f32, tag="qp", name=f"po_{st}_{nh}")
            for c in range(4):
                nc.tensor.matmul(
                    po[:],
                    OT[:, c, st * P:(st + 1) * P],
                    wo[:, c, nh * 512:(nh + 1) * 512],
                    start=(c == 0), stop=(c == 3))
            ob = osbp.tile([P, 512], bf16, tag="ob")
            nc.vector.tensor_copy(ob[:], po[:])
            nc.sync.dma_start(
                outp.ap()[st * P:(st + 1) * P, nh * 512:(nh + 1) * 512],
                ob[:])

        # ---- K projection runs fully up front (also breaks the xin-ring
        # cycle, see above); everything else streams into the attention loop.
        for c in range(4):
            for st in range(4):
                proj_group(wk, bk, xkh, KT, c, st)
        proj_group(wq, bq, xqh, QT, 0, 0)

        # ---- filler bookkeeping ----
        # mandatory[idx] = groups that must complete before iteration idx+1;
        # opportunistic fillers (out-proj of finished q-tiles) drain from a
        # deque, a couple per iteration.
        from collections import deque
        opt_fill = deque()
        ITERS = [(j, p) for j in range(4) for p in range(4)]
        mandatory = {i: [] for i in range(len(ITERS))}
        for idx in range(1, len(ITERS)):
            j, p = ITERS[idx]
            mandatory[idx - 1].append(
                lambda p=p, j=j: proj_group(wq, bq, xqh, QT, p, j))

        for idx, (j, p) in enumerate(ITERS):
            mand = deque(mandatory[idx])

            avs = [pav.tile([HD + 1, 512], f32, tag="av",
                            name=f"av_{j}_{p}_{e}") for e in range(2)]

            def scores(kc):
                st_ = pss.tile([P, 1024], f32, tag="s", name=f"s_{j}_{p}")
                for e in range(2):
                    off = e * HD
                    nc.tensor.matmul(
                        st_[:, e * 512:(e + 1) * 512],
                        KT[off:off + HD, p, kc * P:(kc + 1) * P],
                        QT[off:off + HD, p, j * 512:(j + 1) * 512],
                        start=True, stop=True)
                return st_

            if idx == 0:
                v_group(0)
                v_group(1)
            sts = {0: scores(0), 1: scores(1)}

            for kc in range(16):
                at = attnp.tile([P, 1024], bf16, tag="at")
                nc.scalar.activation(at[:], sts.pop(kc)[:], Exp, scale=0.125)
                # PE filler between exp and this chunk's AV: projections,
                # V JIT (iteration 0), K chunk JIT, out-proj of older q-tiles.
                if idx == 0 and kc + 2 < 16:
                    v_group(kc + 2)
                if mand and kc % 8 == 0:
                    mand.popleft()()
                if opt_fill and kc % 4 == 0:
                    opt_fill.popleft()()
                if kc + 2 < 16:
                    sts[kc + 2] = scores(kc + 2)
                for e in range(2):
                    h = 2 * p + e
                    nc.tensor.matmul(
                        avs[e][:],
                        vaug[:, kc, h * (HD + 1):(h + 1) * (HD + 1)],
                        at[:, e * 512:(e + 1) * 512],
                        start=(kc == 0), stop=(kc == 15))
            while mand:
                mand.popleft()()

            # normalize: row 64 of O' holds the softmax denominators
            for e in range(2):
                off = e * HD
                rc = smallp.tile([1, 512], f32, tag="rc")
                nc.vector.reciprocal(rc[:], avs[e][HD:HD + 1, :])
                bc = smallp.tile([HD, 512], f32, tag="bc")
                nc.gpsimd.partition_broadcast(bc[:], rc[0:1, :])
                nc.vector.tensor_tensor(
                    OT[off:off + HD, p, j * 512:(j + 1) * 512],
                    avs[e][0:HD, :], bc[:], mult)

            if p == 3:
                for st in range(4 * j, 4 * j + 4):
                    for nh in range(2):
                        opt_fill.append(
                            lambda st=st, nh=nh: oproj_group(st, nh))

        while opt_fill:
            opt_fill.popleft()()


def make_in_maps(q, k, v, Wq, bq, Wk, bk, Wv, bv, Wo, bo):
    bf = ml_dtypes.bfloat16
    in_maps = []
    for c in range(N_CORES):
        b_, g = c // 2, c % 2
        sl = slice(g * DG, (g + 1) * DG)
        in_maps.append({
            "xqT": np.ascontiguousarray(q[b_].T).astype(bf),
            "xkT": np.ascontiguousarray(k[b_].T).astype(bf),
            "xvT": np.ascontiguousarray(v[b_].T).astype(bf),
            "wqT": np.ascontiguousarray(Wq[sl].T).astype(bf),
            "wkT": np.ascontiguousarray(Wk[sl].T).astype(bf),
            "wvT": np.ascontiguousarray(Wv[sl].T).astype(bf),
            "woT": np.ascontiguousarray(Wo[:, sl].T).astype(bf),
            "bq": np.ascontiguousarray(
                bq[sl].astype(np.float32).reshape(4, P).T),
            "bk": np.ascontiguousarray(
                bk[sl].astype(np.float32).reshape(4, P).T),
            "bv": np.ascontiguousarray(
                bv[sl].astype(np.float32).reshape(1, DG)),
        })
    return in_maps


def assemble(results, bo):
    out = np.zeros((4, S, DIN), np.float32)
    for b_ in range(4):
        out[b_] = (results[2 * b_]["outp"].astype(np.float32)
                   + results[2 * b_ + 1]["outp"].astype(np.float32))
    out += np.asarray(bo, np.float32)[None, None, :]
    return out


def kernel(q, k, v, Wq, bq, Wk, bk, Wv, bv, Wo, bo):
    from concourse.bass_utils import run_bass_kernel_spmd

    if "nc" not in _CACHE:
        _CACHE["nc"] = build_bass()
    nc = _CACHE["nc"]
    in_maps = make_in_maps(q, k, v, Wq, bq, Wk, bk, Wv, bv, Wo, bo)
    res = run_bass_kernel_spmd(nc, in_maps, core_ids=list(range(N_CORES)))
    return assemble(res.results, bo)


# revision 32
# speedup vs baseline: 1.3653x; 1.3653x over previous
"""Multihead attention (B=4, S=2048, D=1024, H=16) on 8 Trainium2 NeuronCores.

Sharding: data-parallel over batch (4) x tensor-parallel over heads (2 groups
of 8 heads). Core c handles batch c//2, head-group c%2. Q/K/V projections are
column-parallel (each core owns 512 rows of Wq/Wk/Wv), attention is fully
local per head, out-projection is row-parallel (each core owns 512 columns of
Wo) producing a partial [S, D] output; the two partials per batch are summed
on the host (the "all-reduce").

Device layout (per core, all bf16 unless noted):
  xqT/xkT/xvT [1024, 2048]  = x[b].T            (host-transposed, bf16)
  wqT/wkT/wvT [1024, 512]   = W[g*512:,:].T
  woT         [512, 1024]   = Wo[:, g*512:].T
  bq/bk [128, 4] f32 (partition-major), bv [1, 512] f32
  outp        [2048, 1024] f32 (partial output)

On-chip schedule: heads are processed in PAIRS (2p, 2p+1) whose K/Q slices
sit at SBUF partitions 0-63 / 64-127 of chunk p, so the two scores matmuls
(contraction 64) of a pair occupy disjoint row-halves of the PE array
(tile_position (0,0)/(64,0)) and stream concurrently on hardware.

Iteration (j, p): q-tile j (512 q), pair p; inner loop over 16 k-chunks:
  S^T [128k, 1024] = [K_h Q_h^T | K_h' Q_h'^T]  (row-tiled pair, PSUM)
  attn^T = exp(S^T/8) -> SBUF bf16               (ScalarE, the serial floor)
  O'[65, 512] += [V_h | 1]^T attn_h^T  (x2 heads, PSUM accumulate over kc)
Scores are emitted two k-chunks ahead of AV so the PE never waits on the
ScalarE exp. All projection work (K chunks 1-3, Q groups, V groups) and the
out-projection of completed q-tiles are interleaved into the attention loop
as PE filler, so the tensor engine stays continuously busy (which also keeps
it at the 2.4 GHz p-state) and the kernel has no serial projection phases;
only the first iteration (V-projection JIT) and the last q-tile's
out-projection fall outside the steady state.
"""

import sys

if "/opt/trn_rl_repo" not in sys.path:
    sys.path.insert(0, "/opt/trn_rl_repo")

import numpy as np
import ml_dtypes

P = 128
S = 2048
DIN = 1024
DG = 512          # per-core projection width (8 heads * 64)
HD = 64
NH_LOCAL = 8      # heads per core
N_CORES = 8
VA = NH_LOCAL * (HD + 1)  # V_aug free width (520)

_CACHE: dict = {}


def build_bass(repeat: int = 1):
    """Build the SPMD single-core program (same program on all 8 cores)."""
    from concourse import bacc, tile, mybir

    f32 = mybir.dt.float32
    bf16 = mybir.dt.bfloat16

    nc = bacc.Bacc("TRN2", target_bir_lowering=False, debug=False,
                   num_devices=N_CORES)

    xqT = nc.dram_tensor("xqT", [DIN, S], bf16, kind="ExternalInput")
    xkT = nc.dram_tensor("xkT", [DIN, S], bf16, kind="ExternalInput")
    xvT = nc.dram_tensor("xvT", [DIN, S], bf16, kind="ExternalInput")
    wqT = nc.dram_tensor("wqT", [DIN, DG], bf16, kind="ExternalInput")
    wkT = nc.dram_tensor("wkT", [DIN, DG], bf16, kind="ExternalInput")
    wvT = nc.dram_tensor("wvT", [DIN, DG], bf16, kind="ExternalInput")
    woT = nc.dram_tensor("woT", [DG, DIN], bf16, kind="ExternalInput")
    bqd = nc.dram_tensor("bq", [P, 4], f32, kind="ExternalInput")
    bkd = nc.dram_tensor("bk", [P, 4], f32, kind="ExternalInput")
    bvd = nc.dram_tensor("bv", [1, DG], f32, kind="ExternalInput")
    outp = nc.dram_tensor("outp", [S, DIN], f32, kind="ExternalOutput")

    with tile.TileContext(nc) as tc:
        for _ in range(repeat):
            _emit(nc, tc, xqT, xkT, xvT, wqT, wkT, wvT, woT, bqd, bkd, bvd,
                  outp)
    nc.compile()
    return nc


def _emit(nc, tc, xqT, xkT, xvT, wqT, wkT, wvT, woT, bqd, bkd, bvd, outp):
    from concourse import mybir

    f32 = mybir.dt.float32
    bf16 = mybir.dt.bfloat16
    Exp = mybir.ActivationFunctionType.Exp
    mult = mybir.AluOpType.mult
    add_op = mybir.AluOpType.add

    with (
        tc.tile_pool(name="consts", bufs=1) as consts,
        tc.tile_pool(name="xin", bufs=3) as xin,
        tc.tile_pool(name="qkv", bufs=1) as qkvp,
        tc.tile_pool(name="attn", bufs=4) as attnp,
        tc.tile_pool(name="small", bufs=2) as smallp,
        tc.tile_pool(name="osb", bufs=3) as osbp,
        tc.tile_pool(name="ps", bufs=2, space="PSUM") as psp,
        tc.tile_pool(name="pav", bufs=2, space="PSUM") as pav,
        tc.tile_pool(name="pss", bufs=2, space="PSUM") as pss,
    ):
        # PSUM budget (8 banks): "s" [128,1024]f32 x2 = 4 banks (paired
        # scores), "av" [65,512]f32 x2 = 2 banks, "qp" [128,512]f32 x2 =
        # 2 banks (projection + out-proj groups).
        QT = qkvp.tile([P, 4, S], bf16, tag="QT")
        KT = qkvp.tile([P, 4, S], bf16, tag="KT")
        vaug = qkvp.tile([P, 16, VA], bf16, tag="vaug")
        OT = qkvp.tile([P, 4, S], bf16, tag="OT")

        # ones columns of V_aug
        for h in range(NH_LOCAL):
            nc.vector.memset(vaug[:, :, h * (HD + 1) + HD], 1.0)

        def load_x(xdram, tag):
            # x^T [DIN, S] -> two SBUF tiles [P, 4, S]; each half split over
            # two DMA queues (sync + gpsimd) to halve load latency. xk/xq
            # share a 2-slot ring (K projection completes before any Q-proj
            # emission, so the WAR never crosses the attention loop); xv has
            # its own ring because V groups are deferred into iteration 0.
            slot_tag = "xv" if tag == "v" else "xkq"
            xt = xdram.ap().rearrange("(h c p) m -> h p c m", h=2, p=P)
            halves = []
            for hhalf in range(2):
                xh = xin.tile([P, 4, S], bf16, tag=slot_tag, bufs=2)
                nc.sync.dma_start(xh[:, 0:2], xt[hhalf, :, 0:2])
                nc.gpsimd.dma_start(xh[:, 2:4], xt[hhalf, :, 2:4])
                halves.append(xh)
            return halves

        def proj_inputs(xdram, wdram, bdram, wtag):
            bias = consts.tile([P, 4], f32, tag=f"b_{wtag}")
            nc.sync.dma_start(bias[:], bdram.ap())
            w = consts.tile([P, 8, DG], bf16, tag=f"w_{wtag}")
            nc.sync.dma_start(w[:], wdram.ap().rearrange("(c p) m -> p c m",
                                                         p=P))
            halves = load_x(xdram, wtag)
            return w, bias, halves

        def proj_group(w, bias, halves, dstT, c, st):
            # dstT[dq, s] for dq chunk c, s-tile st (one PSUM group)
            pt = psp.tile([P, 512], f32, tag="qp", name=f"pj_{c}_{st}")
            for kc in range(8):
                nc.tensor.matmul(
                    pt[:],
                    w[:, kc, c * P:(c + 1) * P],
                    halves[kc // 4][:, kc % 4, st * 512:(st + 1) * 512],
                    start=(kc == 0),
                    stop=(kc == 7),
                )
            nc.vector.tensor_scalar_add(
                dstT[:, c, st * 512:(st + 1) * 512], pt[:],
                bias[:, c:c + 1])

        # ---- input DMAs + K projection, fully before the xq load so the
        # shared xin ring never creates a PE-order cycle: xq's slot WAR
        # depends on the K-projection matmuls, which all precede any
        # attention work in PE program order.
        wk, bk, xkh = proj_inputs(xkT, wkT, bkd, "k")

        bvrow = consts.tile([1, DG], f32, tag="bvrow")
        nc.sync.dma_start(bvrow[:], bvd.ap())
        bvb = consts.tile([P, DG], f32, tag="bvb")
        nc.gpsimd.partition_broadcast(bvb[:], bvrow[:])
        bvb3 = bvb[:].rearrange("p (h f) -> p h f", f=HD)
        wv = consts.tile([P, 8, DG], bf16, tag="w_v")
        nc.sync.dma_start(wv[:], wvT.ap().rearrange("(c p) m -> p c m", p=P))
        xvh = load_x(xvT, "v")

        wq, bq, xqh = proj_inputs(xqT, wqT, bqd, "q")

        wo = consts.tile([P, 4, DIN], bf16, tag="wo")
        nc.sync.dma_start(wo[:], woT.ap().rearrange("(c p) m -> p c m", p=P))

        def v_group(sc):
            # V projected directly in [s, dv] layout into V_aug columns,
            # bias added via a partition-broadcast row.
            pt = psp.tile([P, DG], f32, tag="qp", name=f"pv_{sc}")
            for kc in range(8):
                nc.tensor.matmul(
                    pt[:],
                    xvh[kc // 4][:, kc % 4, sc * P:(sc + 1) * P],
                    wv[:, kc, :],
                    start=(kc == 0), stop=(kc == 7),
                )
            dst3 = vaug[:, sc].rearrange("p (h f) -> p h f",
                                         f=HD + 1)[:, :, 0:HD]
            src3 = pt[:].rearrange("p (h f) -> p h f", f=HD)
            nc.vector.tensor_tensor(dst3, src3, bvb3, add_op)

        def oproj_group(st, nh):
            # partial[s, dout] = sum_dq OT[dq, s] * woT[dq, dout]
            po = psp.tile([P, 512], f32, tag="qp", name=f"po_{st}_{nh}")
            for c in range(4):
                nc.tensor.matmul(
                    po[:],
                    OT[:, c, st * P:(st + 1) * P],
                    wo[:, c, nh * 512:(nh + 1) * 512],
                    start=(c == 0), stop=(c == 3))
            ob = osbp.tile([P, 512], f32, tag="ob")
            nc.vector.tensor_copy(ob[:], po[:])
            nc.sync.dma_start(
                outp.ap()[st * P:(st + 1) * P, nh * 512:(nh + 1) * 512],
                ob[:])

        # ---- K projection runs fully up front (also breaks the xin-ring
        # cycle, see above); everything else streams into the attention loop.
        for c in range(4):
            for st in range(4):
                proj_group(wk, bk, xkh, KT, c, st)
        proj_group(wq, bq, xqh, QT, 0, 0)

        # ---- filler bookkeeping ----
        # mandatory[idx] = groups that must complete before iteration idx+1;
        # opportunistic fillers (out-proj of finished q-tiles) drain from a
        # deque, a couple per iteration.
        from collections import deque
        opt_fill = deque()
        ITERS = [(j, p) for j in range(4) for p in range(4)]
        mandatory = {i: [] for i in range(len(ITERS))}
        for idx in range(1, len(ITERS)):
            j, p = ITERS[idx]
            mandatory[idx - 1].append(
                lambda p=p, j=j: proj_group(wq, bq, xqh, QT, p, j))

        for idx, (j, p) in enumerate(ITERS):
            mand = deque(mandatory[idx])

            avs = [pav.tile([HD + 1, 512], f32, tag="av",
                            name=f"av_{j}_{p}_{e}") for e in range(2)]

            def scores(kc):
                st_ = pss.tile([P, 1024], f32, tag="s", name=f"s_{j}_{p}")
                for e in range(2):
                    off = e * HD
                    nc.tensor.matmul(
                        st_[:, e * 512:(e + 1) * 512],
                        KT[off:off + HD, p, kc * P:(kc + 1) * P],
                        QT[off:off + HD, p, j * 512:(j + 1) * 512],
                        start=True, stop=True)
                return st_

            if idx == 0:
                v_group(0)
                v_group(1)
            sts = {0: scores(0), 1: scores(1)}

            for kc in range(16):
                at = attnp.tile([P, 1024], bf16, tag="at")
                nc.scalar.activation(at[:], sts.pop(kc)[:], Exp, scale=0.125)
                # PE filler between exp and this chunk's AV: projections,
                # V JIT (iteration 0), out-proj of older q-tiles.
                if idx == 0 and kc + 2 < 16:
                    v_group(kc + 2)
                if mand and kc % 8 == 0:
                    mand.popleft()()
                if opt_fill and kc % 4 == 0:
                    opt_fill.popleft()()
                if kc + 2 < 16:
                    sts[kc + 2] = scores(kc + 2)
                for e in range(2):
                    h = 2 * p + e
                    nc.tensor.matmul(
                        avs[e][:],
                        vaug[:, kc, h * (HD + 1):(h + 1) * (HD + 1)],
                        at[:, e * 512:(e + 1) * 512],
                        start=(kc == 0), stop=(kc == 15))
            while mand:
                mand.popleft()()

            # normalize: row 64 of O' holds the softmax denominators
            for e in range(2):
                off = e * HD
                rc = smallp.tile([1, 512], f32, tag="rc")
                nc.vector.reciprocal(rc[:], avs[e][HD:HD + 1, :])
                bc = smallp.tile([HD, 512], f32, tag="bc")
                nc.gpsimd.partition_broadcast(bc[:], rc[0:1, :])
                nc.vector.tensor_tensor(
                    OT[off:off + HD, p, j * 512:(j + 1) * 512],
                    avs[e][0:HD, :], bc[:], mult)

            if p == 3:
                for st in range(4 * j, 4 * j + 4):
                    for nh in range(2):
                        opt_fill.append(
                            lambda st=st, nh=nh: oproj_group(st, nh))

        while opt_fill:
            opt_fill.popleft()()


def make_in_maps(q, k, v, Wq, bq, Wk, bk, Wv, bv, Wo, bo):
    bf = ml_dtypes.bfloat16
    in_maps = []
    for c in range(N_CORES):
        b_, g = c // 2, c % 2
        sl = slice(g * DG, (g + 1) * DG)
        in_maps.append({
            "xqT": np.ascontiguousarray(q[b_].T).astype(bf),
            "xkT": np.ascontiguousarray(k[b_].T).astype(bf),
            "xvT": np.ascontiguousarray(v[b_].T).astype(bf),
            "wqT": np.ascontiguousarray(Wq[sl].T).astype(bf),
            "wkT": np.ascontiguousarray(Wk[sl].T).astype(bf),
            "wvT": np.ascontiguousarray(Wv[sl].T).astype(bf),
            "woT": np.ascontiguousarray(Wo[:, sl].T).astype(bf),
            "bq": np.ascontiguousarray(
                bq[sl].astype(np.float32).reshape(4, P).T),
            "bk": np.ascontiguousarray(
                bk[sl].astype(np.float32).reshape(4, P).T),
            "bv": np.ascontiguousarray(
                bv[sl].astype(np.float32).reshape(1, DG)),
        })
    return in_maps


def assemble(results, bo):
    out = np.zeros((4, S, DIN), np.float32)
    for b_ in range(4):
        out[b_] = (results[2 * b_]["outp"].astype(np.float32)
                   + results[2 * b_ + 1]["outp"].astype(np.float32))
    out += np.asarray(bo, np.float32)[None, None, :]
    return out


def kernel(q, k, v, Wq, bq, Wk, bk, Wv, bv, Wo, bo):
    from concourse.bass_utils import run_bass_kernel_spmd

    if "nc" not in _CACHE:
        _CACHE["nc"] = build_bass()
    nc = _CACHE["nc"]
    in_maps = make_in_maps(q, k, v, Wq, bq, Wk, bk, Wv, bv, Wo, bo)
    res = run_bass_kernel_spmd(nc, in_maps, core_ids=list(range(N_CORES)))
    return assemble(res.results, bo)
